# revision 15
# baseline (speedup 1.0000x reference)
"""Trainium2 Bass kernel for the DCI loss (bilinear discriminator + BCE).

Strategy (memory-bound; dominated by 204.8 MB of row-gather traffic):
  - Shard the 20 clusters across 8 cores as [3,3,3,3,2,2,2,2]; cores with
    2 real clusters process a dummy 3rd slot whose output is discarded.
  - Per core: gather its clusters' h_1 rows into SBUF (kept resident),
    compute per-cluster column sums on the TensorEngine (ones-matmul into
    PSUM), sigmoid -> c, Wc = W @ c via small matmuls, replicate Wc across
    partitions, then score rows with fused multiply-reduce on the
    VectorEngine.  h_2 rows are streamed through a small pool.
  - softplus(+/-(score+b)) and per-slot sums via ScalarEngine accum_out;
    final cross-partition reduction via a ones-matmul.  Host combines the
    8x6 per-slot sums into the scalar mean.
  - Indices are sorted per cluster on the host so the gather walks HBM
    nearly monotonically.
"""

import sys

if "/opt/trn_rl_repo" not in sys.path:
    sys.path.insert(0, "/opt/trn_rl_repo")

import numpy as np

import concourse.bass as bass
import concourse.tile as tile
from concourse import bacc, mybir
from concourse.bass import IndirectOffsetOnAxis
from concourse.bass_utils import run_bass_kernel_spmd

# Problem constants (hardcoded per the task contract).
N_NODES = 100000
D = 256
K_CLUSTERS = 20
CLUSTER_SIZE = 5000
N_CORES = 8

F32 = mybir.dt.float32
I32 = mybir.dt.int32


def build_program(n_nodes, cs, slots, p, n_cores):
    """Build the SPMD Bass program. cs = cluster size, p = rows per tile."""
    tpc = cs // p          # tiles per cluster slot
    tps = slots * tpc      # tiles per core per h-tensor
    assert tpc * p == cs

    nc = bacc.Bacc("TRN2", target_bir_lowering=False, debug=False,
                   num_devices=n_cores)

    h1 = nc.dram_tensor("h1", [n_nodes, D], F32, kind="ExternalInput").ap()
    h2 = nc.dram_tensor("h2", [n_nodes, D], F32, kind="ExternalInput").ap()
    idx = nc.dram_tensor("idx", [p, tps], I32, kind="ExternalInput").ap()
    wt = nc.dram_tensor("wt", [D, D], F32, kind="ExternalInput").ap()  # W.T
    ones_col = nc.dram_tensor("ones_col", [p, 1], F32, kind="ExternalInput").ap()
    ones_row = nc.dram_tensor("ones_row", [1, p], F32, kind="ExternalInput").ap()
    bpos = nc.dram_tensor("bpos", [p, 1], F32, kind="ExternalInput").ap()
    bneg = nc.dram_tensor("bneg", [p, 1], F32, kind="ExternalInput").ap()
    out = nc.dram_tensor("out", [1, 2 * slots], F32, kind="ExternalOutput").ap()

    with tile.TileContext(nc) as tc:
        with (
            tc.tile_pool(name="const", bufs=1) as cpool,
            tc.tile_pool(name="h1res", bufs=1) as h1pool,
            tc.tile_pool(name="h2s", bufs=16) as h2pool,
            tc.tile_pool(name="wc", bufs=1) as wcpool,
            tc.tile_pool(name="sc", bufs=1) as scpool,
            tc.tile_pool(name="scr", bufs=4) as scrpool,
            tc.tile_pool(name="psS", bufs=slots, space="PSUM") as psS,
            tc.tile_pool(name="psSmall", bufs=2, space="PSUM") as psSmall,
            tc.tile_pool(name="psRep", bufs=2, space="PSUM") as psRep,
        ):
            # ---- constants ----
            idx_sb = cpool.tile([p, tps], I32)
            nc.sync.dma_start(idx_sb[:], idx[:])
            wt_sb = cpool.tile([128, 2 * D], F32)  # [e0:wt rows 0..127 | e1]
            nc.sync.dma_start(wt_sb[:, 0:D], wt[0:128, :])
            nc.sync.dma_start(wt_sb[:, D:2 * D], wt[128:256, :])
            ones_c = cpool.tile([p, 1], F32)
            nc.sync.dma_start(ones_c[:], ones_col[:])
            ones_r = cpool.tile([1, p], F32)
            nc.sync.dma_start(ones_r[:], ones_row[:])
            bpos_sb = cpool.tile([p, 1], F32)
            nc.sync.dma_start(bpos_sb[:], bpos[:])
            bneg_sb = cpool.tile([p, 1], F32)
            nc.sync.dma_start(bneg_sb[:], bneg[:])

            # score tile: prod = x * WcRep on DVE, then row-sum via ACT
            # accum_out (tensor_tensor_reduce crashes the exec unit on HW).
            def score_tile(x_ap, j, acc_ap, uid):
                prod = scrpool.tile([p, D], F32, tag="prod", name=f"pr{uid}")
                nc.vector.tensor_tensor(out=prod[:], in0=x_ap,
                                        in1=wcrep[j][:],
                                        op=mybir.AluOpType.mult)
                scr2 = scrpool.tile([p, D], F32, tag="scr2", name=f"s2{uid}")
                nc.scalar.activation(
                    out=scr2[:], in_=prod[:],
                    func=mybir.ActivationFunctionType.Copy,
                    accum_out=acc_ap,
                )

            # ---- persistent buffers ----
            h1big = h1pool.tile([p, tps * D], F32)
            sc_all = scpool.tile([p, 2 * tps], F32)   # [h1 cols | h2 cols]
            accs = scpool.tile([p, 2 * slots], F32)   # per-slot softplus sums
            wcrep = [wcpool.tile([p, D], F32, tag=f"wcrep{j}", name=f"wcrep{j}")
                     for j in range(slots)]
            c_sb = wcpool.tile([1, slots * D], F32)
            wc_sb = wcpool.tile([1, slots * D], F32)

            # ---- phase A: gather h1, per-slot column sums, Wc chain ----
            for j in range(slots):
                psum_S = psS.tile([1, D], F32, space="PSUM", tag="S",
                                  name=f"S{j}")
                for i in range(tpc):
                    t = j * tpc + i
                    nc.gpsimd.indirect_dma_start(
                        out=h1big[:, t * D:(t + 1) * D],
                        out_offset=None,
                        in_=h1[:],
                        in_offset=IndirectOffsetOnAxis(ap=idx_sb[:, t:t + 1],
                                                       axis=0),
                    )
                    nc.tensor.matmul(
                        out=psum_S[:],
                        lhsT=ones_c[:],
                        rhs=h1big[:, t * D:(t + 1) * D],
                        start=(i == 0),
                        stop=(i == tpc - 1),
                    )
                # c_j = sigmoid(S_j / cs) = 1 / (1 + exp(-S_j / cs))
                # (exp/ln/abs/relu share one ACT table; Sigmoid does not)
                ce = scrpool.tile([1, D], F32, tag="ce")
                nc.scalar.activation(
                    out=ce[:],
                    in_=psum_S[:],
                    func=mybir.ActivationFunctionType.Exp,
                    scale=-1.0 / cs,
                )
                cep1 = scrpool.tile([1, D], F32, tag="cep1")
                nc.vector.tensor_scalar_add(cep1[:], ce[:], 1.0)
                nc.vector.reciprocal(c_sb[:, j * D:(j + 1) * D], cep1[:])
                # transpose c_j -> [D, 1] (two 128-halves) via PE
                ct_ps = psSmall.tile([128, 2], F32, space="PSUM", tag="ps_sm")
                nc.tensor.transpose(
                    out=ct_ps[:, 0:1],
                    in_=c_sb[:, j * D:j * D + 128],
                    identity=ones_r[:, 0:1],
                )
                nc.tensor.transpose(
                    out=ct_ps[:, 1:2],
                    in_=c_sb[:, j * D + 128:(j + 1) * D],
                    identity=ones_r[:, 0:1],
                )
                ct_sb = scrpool.tile([128, 2], F32, tag="ct")
                nc.vector.tensor_copy(ct_sb[:], ct_ps[:])
                # Wc_j[d] = sum_e wt[e, d] * c_j[e]   -> [1, D]
                wc_ps = psSmall.tile([1, D], F32, space="PSUM", tag="ps_sm")
                nc.tensor.matmul(out=wc_ps[:], lhsT=ct_sb[:, 0:1],
                                 rhs=wt_sb[:, 0:D], start=True, stop=False)
                nc.tensor.matmul(out=wc_ps[:], lhsT=ct_sb[:, 1:2],
                                 rhs=wt_sb[:, D:2 * D], start=False, stop=True)
                nc.vector.tensor_copy(wc_sb[:, j * D:(j + 1) * D], wc_ps[:])
                # replicate Wc_j across p partitions
                rep_ps = psRep.tile([p, D], F32, space="PSUM", tag="rep")
                nc.tensor.matmul(out=rep_ps[:], lhsT=ones_r[:],
                                 rhs=wc_sb[:, j * D:(j + 1) * D],
                                 start=True, stop=True)
                nc.vector.tensor_copy(wcrep[j][:], rep_ps[:])

                # score this slot's h1 tiles as soon as Wc_j is ready
                for i in range(tpc):
                    t = j * tpc + i
                    score_tile(h1big[:, t * D:(t + 1) * D], j,
                               sc_all[:, t:t + 1], f"a{t}")

            # ---- phase B: stream h2, score ----
            for t in range(tps):
                j = t // tpc
                h2t = h2pool.tile([p, D], F32, tag="h2t")
                nc.gpsimd.indirect_dma_start(
                    out=h2t[:],
                    out_offset=None,
                    in_=h2[:],
                    in_offset=IndirectOffsetOnAxis(ap=idx_sb[:, t:t + 1],
                                                   axis=0),
                )
                score_tile(h2t[:], j, sc_all[:, tps + t:tps + t + 1], f"b{t}")

            # ---- phase C: softplus + per-slot sums + final reduce ----
            # x = sgn*(sc + b); softplus(x) = relu(x) + ln(1 + exp(-|x|)).
            # |x| = |sc + b| regardless of sgn; relu bias is sgn*b.
            def softplus_sum(in_ap, sgn, acc_ap, uid):
                av = scrpool.tile([p, tpc], F32, tag="spa", name=f"spa{uid}")
                nc.scalar.activation(
                    out=av[:], in_=in_ap,
                    func=mybir.ActivationFunctionType.Abs,
                    bias=bpos_sb[:], scale=1.0)
                ev = scrpool.tile([p, tpc], F32, tag="spe", name=f"spe{uid}")
                nc.scalar.activation(
                    out=ev[:], in_=av[:],
                    func=mybir.ActivationFunctionType.Exp, scale=-1.0)
                lv = scrpool.tile([p, tpc], F32, tag="spl", name=f"spl{uid}")
                nc.scalar.activation(
                    out=lv[:], in_=ev[:],
                    func=mybir.ActivationFunctionType.Ln, bias=1.0, scale=1.0)
                rv = scrpool.tile([p, tpc], F32, tag="spr", name=f"spr{uid}")
                nc.scalar.activation(
                    out=rv[:], in_=in_ap,
                    func=mybir.ActivationFunctionType.Relu,
                    bias=(bpos_sb[:] if sgn > 0 else bneg_sb[:]), scale=sgn)
                sv = scrpool.tile([p, tpc], F32, tag="spv", name=f"spv{uid}")
                nc.vector.tensor_add(out=sv[:], in0=lv[:], in1=rv[:])
                s2 = scrpool.tile([p, tpc], F32, tag="spw", name=f"spw{uid}")
                nc.scalar.activation(
                    out=s2[:], in_=sv[:],
                    func=mybir.ActivationFunctionType.Copy,
                    accum_out=acc_ap)

            for j in range(slots):
                softplus_sum(sc_all[:, j * tpc:(j + 1) * tpc], -1.0,
                             accs[:, 2 * j:2 * j + 1], f"h1_{j}")
                softplus_sum(sc_all[:, tps + j * tpc:tps + (j + 1) * tpc], 1.0,
                             accs[:, 2 * j + 1:2 * j + 2], f"h2_{j}")
            out_ps = psSmall.tile([1, 2 * slots], F32, space="PSUM", tag="ps_sm")
            nc.tensor.matmul(out=out_ps[:], lhsT=ones_c[:], rhs=accs[:],
                             start=True, stop=True)
            out_sb = scrpool.tile([1, 2 * slots], F32, tag="osb")
            nc.vector.tensor_copy(out_sb[:], out_ps[:])
            nc.sync.dma_start(out[:], out_sb[:])

    nc.compile()
    return nc


def make_assignment(k_clusters, n_cores, slots):
    """Cluster ids per core; dummy slots repeat the core's first cluster."""
    base = k_clusters // n_cores
    extra = k_clusters % n_cores
    assign, pos = [], 0
    for c in range(n_cores):
        n = base + (1 if c < extra else 0)
        ids = list(range(pos, pos + n))
        pos += n
        real = len(ids)
        while len(ids) < slots:
            ids.append(ids[0] if ids else 0)
        assign.append((ids, real))
    assert pos == k_clusters
    return assign


def make_idx_array(cluster_info, ids, cs, p):
    """[p, slots*tpc] int32: tile t's indices = sorted cluster block."""
    tpc = cs // p
    cols = []
    for k in ids:
        srt = np.sort(cluster_info[k].astype(np.int64)).astype(np.int32)
        cols.append(srt.reshape(tpc, p).T)  # [p, tpc]
    return np.ascontiguousarray(np.concatenate(cols, axis=1))


_CACHE = {}


def _get_program(key, *args):
    if key not in _CACHE:
        _CACHE[key] = build_program(*args)
    return _CACHE[key]


def run(h_1, h_2, cluster_info, W, b,
        n_nodes=N_NODES, cs=CLUSTER_SIZE, k_clusters=K_CLUSTERS,
        n_cores=N_CORES, slots=3, p=125, **spmd_kwargs):
    h_1 = np.ascontiguousarray(np.asarray(h_1, dtype=np.float32))
    h_2 = np.ascontiguousarray(np.asarray(h_2, dtype=np.float32))
    ci = np.asarray(cluster_info).reshape(k_clusters, cs)
    W = np.asarray(W, dtype=np.float32).reshape(D, D)
    bval = float(np.asarray(b))

    nc = _get_program((n_nodes, cs, slots, p, n_cores),
                      n_nodes, cs, slots, p, n_cores)

    assign = make_assignment(k_clusters, n_cores, slots)
    wt = np.ascontiguousarray(W.T)
    ones_col = np.ones((p, 1), np.float32)
    ones_row = np.ones((1, p), np.float32)
    bpos = np.full((p, 1), bval, np.float32)
    bneg = np.full((p, 1), -bval, np.float32)

    in_maps = []
    for c in range(n_cores):
        ids, _ = assign[c]
        in_maps.append({
            "h1": h_1, "h2": h_2,
            "idx": make_idx_array(ci, ids, cs, p),
            "wt": wt, "ones_col": ones_col, "ones_row": ones_row,
            "bpos": bpos, "bneg": bneg,
        })

    res = run_bass_kernel_spmd(nc, in_maps, core_ids=list(range(n_cores)),
                               **spmd_kwargs)

    per_cluster = np.zeros(k_clusters, np.float64)
    for c in range(n_cores):
        ids, real = assign[c]
        o = np.asarray(res.results[c]["out"], np.float64).reshape(2 * slots)
        for j in range(real):
            per_cluster[ids[j]] = (o[2 * j] + o[2 * j + 1]) / (2.0 * cs)
    return np.float32(per_cluster.mean()), res


def kernel(h_1, h_2, cluster_info, W, b):
    val, _ = run(h_1, h_2, cluster_info, W, b)
    return np.array(val, dtype=np.float32)


# revision 18
# speedup vs baseline: 2.7831x; 2.7831x over previous
"""Trainium2 Bass kernel for the DCI loss (bilinear discriminator + BCE).

Strategy (memory-bound; dominated by 204.8 MB of row-gather traffic):
  - Shard the 20 clusters across 8 cores as [3,3,3,3,2,2,2,2]; cores with
    2 real clusters process a dummy 3rd slot whose output is discarded.
  - Rows are gathered 128 per tile via wide indirect DMAs (many tiles of
    indices per INDIRECT1D instruction -- the per-instruction GpSimd
    descriptor-generation cost is ~1.3us regardless of width).  Clusters
    (5000 rows) are padded to 5120 rows with indices pointing at a
    host-appended all-zero row; the host subtracts the known
    softplus(+/-b) contribution of the 120 pad rows per cluster.
  - Gathered data is cast to bf16 in the DMA (HBM traffic unchanged,
    on-chip cost halved).  h_1 stays SBUF-resident (reused for the
    cluster mean and the scores); h_2 is streamed.
  - Cluster summary: TensorEngine ones-matmul column sums (bf16 in, f32
    PSUM), sigmoid via exp+reciprocal (exp/ln/abs/relu share one ACT
    table), Wc = W @ c via small f32 matmuls, replicated across
    partitions with a rank-1 matmul.
  - Scores: batched DVE multiplies against a broadcast-AP of Wc, then
    per-tile row-sum reduces split between DVE (3D segmented reduce_sum)
    and ACT (Copy with accum_out) to balance engine load.
    (tensor_tensor_reduce would fuse these but crashes the exec unit.)
  - softplus(x) = relu(x) + ln(1+exp(-|x|)) on ACT; per-slot sums via
    accum_out; final cross-partition reduce via a ones-matmul.  Host
    combines the 8x6 per-slot sums into the scalar mean.
  - Indices are sorted per cluster on the host so the gather walks HBM
    nearly monotonically.
"""

import sys

if "/opt/trn_rl_repo" not in sys.path:
    sys.path.insert(0, "/opt/trn_rl_repo")

import numpy as np

import concourse.bass as bass
import concourse.tile as tile
from concourse import bacc, mybir
from concourse.bass import IndirectOffsetOnAxis
from concourse.bass_utils import run_bass_kernel_spmd

# Problem constants (hardcoded per the task contract).
N_NODES = 100000
D = 256
K_CLUSTERS = 20
CLUSTER_SIZE = 5000
N_CORES = 8

P = 128                      # rows per gather tile (partition dim)
F32 = mybir.dt.float32
BF16 = mybir.dt.bfloat16
I32 = mybir.dt.int32
AF = mybir.ActivationFunctionType
ALU = mybir.AluOpType


def build_program(n_nodes, cs, tpc, slots, n_cores, gw, dve_frac=0.4):
    """SPMD Bass program.

    tpc: tiles (of P rows) per cluster slot; gw: gather width in tiles
    per indirect DMA; dve_frac: fraction of score reduces done on DVE
    (rest on ACT).
    """
    tps = slots * tpc        # tiles per core per h-tensor
    nc = bacc.Bacc("TRN2", target_bir_lowering=False, debug=False,
                   num_devices=n_cores)

    # h tables carry one extra all-zero row used by pad indices.
    h1 = nc.dram_tensor("h1", [n_nodes + 1, D], F32, kind="ExternalInput").ap()
    h2 = nc.dram_tensor("h2", [n_nodes + 1, D], F32, kind="ExternalInput").ap()
    idx = nc.dram_tensor("idx", [P, tps], I32, kind="ExternalInput").ap()
    wt = nc.dram_tensor("wt", [D, D], F32, kind="ExternalInput").ap()  # W.T
    ones_bf = nc.dram_tensor("ones_bf", [P, 1], F32, kind="ExternalInput").ap()
    ones_f = nc.dram_tensor("ones_f", [P, 1], F32, kind="ExternalInput").ap()
    ones_r = nc.dram_tensor("ones_r", [1, P], F32, kind="ExternalInput").ap()
    bpos = nc.dram_tensor("bpos", [P, 1], F32, kind="ExternalInput").ap()
    bneg = nc.dram_tensor("bneg", [P, 1], F32, kind="ExternalInput").ap()
    out = nc.dram_tensor("out", [1, 2 * slots], F32, kind="ExternalOutput").ap()

    n_chunks = (tpc + gw - 1) // gw

    with tile.TileContext(nc) as tc:
        with (
            tc.tile_pool(name="const", bufs=1) as cpool,
            tc.tile_pool(name="h1res", bufs=1) as h1pool,
            tc.tile_pool(name="h2s", bufs=6) as h2pool,
            tc.tile_pool(name="wc", bufs=1) as wcpool,
            tc.tile_pool(name="sc", bufs=1) as scpool,
            tc.tile_pool(name="scr", bufs=4) as scrpool,
            tc.tile_pool(name="psS", bufs=slots, space="PSUM") as psS,
            tc.tile_pool(name="psSmall", bufs=2, space="PSUM") as psSmall,
            tc.tile_pool(name="psRep", bufs=2, space="PSUM") as psRep,
        ):
            # ---- constants ----
            idx_sb = cpool.tile([P, tps], I32)
            nc.sync.dma_start(idx_sb[:], idx[:])
            wt_sb = cpool.tile([128, 2 * D], F32)  # [e rows 0..127 | 128..255]
            nc.sync.dma_start(wt_sb[:, 0:D], wt[0:128, :])
            nc.sync.dma_start(wt_sb[:, D:2 * D], wt[128:256, :])
            ones_b = cpool.tile([P, 1], BF16)
            nc.gpsimd.dma_start(ones_b[:], ones_bf[:])   # f32 -> bf16 cast
            ones_c = cpool.tile([P, 1], F32)
            nc.sync.dma_start(ones_c[:], ones_f[:])
            ones_rw = cpool.tile([1, P], F32)
            nc.sync.dma_start(ones_rw[:], ones_r[:])
            bpos_sb = cpool.tile([P, 1], F32)
            nc.sync.dma_start(bpos_sb[:], bpos[:])
            bneg_sb = cpool.tile([P, 1], F32)
            nc.sync.dma_start(bneg_sb[:], bneg[:])

            # ---- persistent buffers ----
            h1big = h1pool.tile([P, tps * D], BF16)
            sc_all = scpool.tile([P, 2 * tps], F32)   # [h1 cols | h2 cols]
            accs = scpool.tile([P, 2 * slots], F32)
            wcrep = [wcpool.tile([P, D], BF16, tag=f"wcrep{j}",
                                 name=f"wcrep{j}") for j in range(slots)]
            c_sb = wcpool.tile([1, slots * D], F32)
            wc_sb = wcpool.tile([1, slots * D], F32)

            def gather(dst_ap, src, c0, c1):
                nc.gpsimd.indirect_dma_start(
                    out=dst_ap, out_offset=None, in_=src,
                    in_offset=IndirectOffsetOnAxis(ap=idx_sb[:, c0:c1],
                                                   axis=0))

            rcount = [0]

            def score_chunk(src_ap, w, wc_bc, sc_ap, uid):
                """sc_ap[:, 0:w] = row-sums of (src*wcrep_j) per tile."""
                prod = scrpool.tile([P, gw * D], BF16, tag="prod",
                                    name=f"pr{uid}")
                pr = prod[:, 0:w * D].rearrange("p (w d) -> p w d", w=w)
                nc.vector.tensor_tensor(
                    out=pr, in0=src_ap.rearrange("p (w d) -> p w d", w=w),
                    in1=wc_bc, op=ALU.mult)
                for u in range(w):
                    rcount[0] += 1
                    if (rcount[0] * dve_frac) % 1 < dve_frac:
                        nc.vector.reduce_sum(
                            sc_ap[:, u:u + 1],
                            prod[:, u * D:(u + 1) * D],
                            axis=mybir.AxisListType.X)
                    else:
                        s2 = scrpool.tile([P, D], BF16, tag="scr2",
                                          name=f"s2{uid}_{u}")
                        nc.scalar.activation(
                            out=s2[:], in_=prod[:, u * D:(u + 1) * D],
                            func=AF.Copy,
                            accum_out=sc_ap[:, u:u + 1])

            # ---- per-slot: gather h1, column sums, Wc chain, h1 scores ----
            for j in range(slots):
                psum_S = psS.tile([1, D], F32, space="PSUM", tag="S",
                                  name=f"S{j}")
                for ci in range(n_chunks):
                    c0, c1 = ci * gw, min((ci + 1) * gw, tpc)
                    t0, t1 = j * tpc + c0, j * tpc + c1
                    gather(h1big[:, t0 * D:t1 * D], h1[:], t0, t1)
                    for t in range(t0, t1):
                        nc.tensor.matmul(
                            out=psum_S[:],
                            lhsT=ones_b[:],
                            rhs=h1big[:, t * D:(t + 1) * D],
                            start=(t == j * tpc),
                            stop=(t == j * tpc + tpc - 1))
                # c_j = 1 / (1 + exp(-S_j / cs))
                ce = scrpool.tile([1, D], F32, tag="ce", name=f"ce{j}")
                nc.scalar.activation(out=ce[:], in_=psum_S[:], func=AF.Exp,
                                     scale=-1.0 / cs)
                cep1 = scrpool.tile([1, D], F32, tag="cep1", name=f"cp{j}")
                nc.vector.tensor_scalar_add(cep1[:], ce[:], 1.0)
                nc.vector.reciprocal(c_sb[:, j * D:(j + 1) * D], cep1[:])
                # transpose c_j -> [D, 1] halves via PE
                ct_ps = psSmall.tile([128, 2], F32, space="PSUM", tag="ps_sm",
                                     name=f"ctp{j}")
                nc.tensor.transpose(out=ct_ps[:, 0:1],
                                    in_=c_sb[:, j * D:j * D + 128],
                                    identity=ones_rw[:, 0:1])
                nc.tensor.transpose(out=ct_ps[:, 1:2],
                                    in_=c_sb[:, j * D + 128:(j + 1) * D],
                                    identity=ones_rw[:, 0:1])
                ct_sb = scrpool.tile([128, 2], F32, tag="ct", name=f"ct{j}")
                nc.vector.tensor_copy(ct_sb[:], ct_ps[:])
                # Wc_j[d] = sum_e wt[e, d] * c_j[e]
                wc_ps = psSmall.tile([1, D], F32, space="PSUM", tag="ps_sm",
                                     name=f"wcp{j}")
                nc.tensor.matmul(out=wc_ps[:], lhsT=ct_sb[:, 0:1],
                                 rhs=wt_sb[:, 0:D], start=True, stop=False)
                nc.tensor.matmul(out=wc_ps[:], lhsT=ct_sb[:, 1:2],
                                 rhs=wt_sb[:, D:2 * D], start=False, stop=True)
                nc.vector.tensor_copy(wc_sb[:, j * D:(j + 1) * D], wc_ps[:])
                # replicate Wc_j across partitions; store bf16
                rep_ps = psRep.tile([P, D], F32, space="PSUM", tag="rep",
                                    name=f"rp{j}")
                nc.tensor.matmul(out=rep_ps[:], lhsT=ones_rw[:],
                                 rhs=wc_sb[:, j * D:(j + 1) * D],
                                 start=True, stop=True)
                nc.vector.tensor_copy(wcrep[j][:], rep_ps[:])

                # h1 scores for this slot
                for ci in range(n_chunks):
                    c0, c1 = ci * gw, min((ci + 1) * gw, tpc)
                    t0 = j * tpc + c0
                    w = c1 - c0
                    wc_bc = wcrep[j][:].rearrange(
                        "p (w d) -> p w d", w=1).to_broadcast([P, w, D])
                    score_chunk(h1big[:, t0 * D:(t0 + w) * D], w, wc_bc,
                                sc_all[:, t0:t0 + w], f"a{j}_{ci}")

            # ---- stream h2, score ----
            for j in range(slots):
                for ci in range(n_chunks):
                    c0, c1 = ci * gw, min((ci + 1) * gw, tpc)
                    t0 = j * tpc + c0
                    w = c1 - c0
                    h2t = h2pool.tile([P, gw * D], BF16, tag="h2t",
                                      name=f"h2t{j}_{ci}")
                    gather(h2t[:, 0:w * D], h2[:], t0, t0 + w)
                    wc_bc = wcrep[j][:].rearrange(
                        "p (w d) -> p w d", w=1).to_broadcast([P, w, D])
                    score_chunk(h2t[:, 0:w * D], w, wc_bc,
                                sc_all[:, tps + t0:tps + t0 + w],
                                f"b{j}_{ci}")

            # ---- softplus + per-slot sums + final reduce ----
            # x = sgn*(sc + b); softplus(x) = relu(x) + ln(1 + exp(-|x|)).
            def softplus_sum(in_ap, sgn, acc_ap, uid):
                av = scrpool.tile([P, tpc], F32, tag="spa", name=f"spa{uid}")
                nc.scalar.activation(out=av[:], in_=in_ap, func=AF.Abs,
                                     bias=bpos_sb[:], scale=1.0)
                ev = scrpool.tile([P, tpc], F32, tag="spe", name=f"spe{uid}")
                nc.scalar.activation(out=ev[:], in_=av[:], func=AF.Exp,
                                     scale=-1.0)
                lv = scrpool.tile([P, tpc], F32, tag="spl", name=f"spl{uid}")
                nc.scalar.activation(out=lv[:], in_=ev[:], func=AF.Ln,
                                     bias=1.0, scale=1.0)
                rv = scrpool.tile([P, tpc], F32, tag="spr", name=f"spr{uid}")
                nc.scalar.activation(out=rv[:], in_=in_ap, func=AF.Relu,
                                     bias=(bpos_sb[:] if sgn > 0
                                           else bneg_sb[:]), scale=sgn)
                sv = scrpool.tile([P, tpc], F32, tag="spv", name=f"spv{uid}")
                nc.vector.tensor_add(out=sv[:], in0=lv[:], in1=rv[:])
                s2 = scrpool.tile([P, tpc], F32, tag="spw", name=f"spw{uid}")
                nc.scalar.activation(out=s2[:], in_=sv[:], func=AF.Copy,
                                     accum_out=acc_ap)

            for j in range(slots):
                softplus_sum(sc_all[:, j * tpc:(j + 1) * tpc], -1.0,
                             accs[:, 2 * j:2 * j + 1], f"h1_{j}")
                softplus_sum(sc_all[:, tps + j * tpc:tps + (j + 1) * tpc], 1.0,
                             accs[:, 2 * j + 1:2 * j + 2], f"h2_{j}")
            out_ps = psSmall.tile([1, 2 * slots], F32, space="PSUM",
                                  tag="ps_sm", name="outp")
            nc.tensor.matmul(out=out_ps[:], lhsT=ones_c[:], rhs=accs[:],
                             start=True, stop=True)
            out_sb = scrpool.tile([1, 2 * slots], F32, tag="osb", name="osb")
            nc.vector.tensor_copy(out_sb[:], out_ps[:])
            nc.sync.dma_start(out[:], out_sb[:])

    nc.compile()
    return nc


def make_assignment(k_clusters, n_cores, slots):
    """Cluster ids per core; dummy slots repeat the core's first cluster."""
    base = k_clusters // n_cores
    extra = k_clusters % n_cores
    assign, pos = [], 0
    for c in range(n_cores):
        n = base + (1 if c < extra else 0)
        ids = list(range(pos, pos + n))
        pos += n
        real = len(ids)
        while len(ids) < slots:
            ids.append(ids[0] if ids else 0)
        assign.append((ids, real))
    assert pos == k_clusters
    return assign


def make_idx_array(cluster_info, ids, cs, tpc, pad_index):
    """[P, slots*tpc] int32; tile t = consecutive sorted block, pads last."""
    cols = []
    for k in ids:
        srt = np.sort(cluster_info[k].astype(np.int64)).astype(np.int32)
        padded = np.concatenate(
            [srt, np.full(tpc * P - cs, pad_index, np.int32)])
        cols.append(padded.reshape(tpc, P).T)  # [P, tpc]
    return np.ascontiguousarray(np.concatenate(cols, axis=1))


_CACHE = {}


def _get_program(key, *args, **kw):
    if key not in _CACHE:
        _CACHE[key] = build_program(*args, **kw)
    return _CACHE[key]


def _np_softplus(x):
    return np.log1p(np.exp(-abs(x))) + max(x, 0.0)


def run(h_1, h_2, cluster_info, W, b,
        n_nodes=N_NODES, cs=CLUSTER_SIZE, k_clusters=K_CLUSTERS,
        n_cores=N_CORES, slots=3, gw=10, **spmd_kwargs):
    h_1 = np.asarray(h_1, dtype=np.float32)
    h_2 = np.asarray(h_2, dtype=np.float32)
    ci = np.asarray(cluster_info).reshape(k_clusters, cs)
    W = np.asarray(W, dtype=np.float32).reshape(D, D)
    bval = float(np.asarray(b))

    tpc = (cs + P - 1) // P
    gw = min(gw, tpc)
    n_pad = tpc * P - cs

    nc = _get_program((n_nodes, cs, tpc, slots, n_cores, gw),
                      n_nodes, cs, tpc, slots, n_cores, gw)

    zrow = np.zeros((1, D), np.float32)
    h1x = np.ascontiguousarray(np.concatenate([h_1, zrow], axis=0))
    h2x = np.ascontiguousarray(np.concatenate([h_2, zrow], axis=0))

    assign = make_assignment(k_clusters, n_cores, slots)
    wt = np.ascontiguousarray(W.T)
    onesP = np.ones((P, 1), np.float32)
    ones_row = np.ones((1, P), np.float32)
    bpos = np.full((P, 1), bval, np.float32)
    bneg = np.full((P, 1), -bval, np.float32)

    in_maps = []
    for c in range(n_cores):
        ids, _ = assign[c]
        in_maps.append({
            "h1": h1x, "h2": h2x,
            "idx": make_idx_array(ci, ids, cs, tpc, n_nodes),
            "wt": wt, "ones_bf": onesP, "ones_f": onesP,
            "ones_r": ones_row, "bpos": bpos, "bneg": bneg,
        })

    res = run_bass_kernel_spmd(nc, in_maps, core_ids=list(range(n_cores)),
                               **spmd_kwargs)

    # Pad rows score exactly b; subtract their known softplus contribution.
    pad1 = n_pad * _np_softplus(-bval)
    pad2 = n_pad * _np_softplus(bval)
    per_cluster = np.zeros(k_clusters, np.float64)
    for c in range(n_cores):
        ids, real = assign[c]
        o = np.asarray(res.results[c]["out"], np.float64).reshape(2 * slots)
        for j in range(real):
            per_cluster[ids[j]] = ((o[2 * j] - pad1) +
                                   (o[2 * j + 1] - pad2)) / (2.0 * cs)
    return np.float32(per_cluster.mean()), res


def kernel(h_1, h_2, cluster_info, W, b):
    val, _ = run(h_1, h_2, cluster_info, W, b)
    return np.array(val, dtype=np.float32)
